# revision 5
# baseline (speedup 1.0000x reference)
"""Trainium2 Bass kernel for CustomTradingLoss.

Computes, over B=8388608 samples with C=3 classes:
    ce      = logsumexp(pred) - pred[target]          (per sample)
    loss    = 0.85 * mean(ce * |pc|) / (mean(|pc|) + 1e-8)
            + 0.15 * mean(ce)
            + 0.1  * mean(where(aligned, -0.1, 0))
    aligned = (td > 0 & t == 2) | (td < 0 & t == 0)  == ((t-1)*td > 0)

Pure data parallel across 8 NeuronCores: core c gets samples
[c*B/8, (c+1)*B/8), laid out [128 partitions x 8192 free]. Each core
emits per-tile partial sums (Sce, Sw, Sap, Sal) as [128, n_tiles]
columns; the host reduces them in f64 and applies the final formula.
"""

import os
import sys

import numpy as np

for _p in ("/opt/trn_rl_repo", "/opt/trn_rl_repo/concourse"):
    if os.path.isdir(_p) and _p not in sys.path:
        sys.path.insert(0, _p)

import concourse.bacc as bacc
import concourse.mybir as mybir
import concourse.tile as tile
from concourse.bass_utils import run_bass_kernel_spmd

B = 8388608
C = 3
N_CORES = 8
N_PER_CORE = B // N_CORES  # 1048576
P = 128
F = N_PER_CORE // P  # 8192 free elements per partition

DIRECTIONAL_WEIGHT = 0.85
MAGNITUDE_WEIGHT = 0.15
TREND_WEIGHT = 0.1
EPS = 1e-8

f32 = mybir.dt.float32
AF = mybir.ActivationFunctionType
OP = mybir.AluOpType


def build(p=P, f=F, t=1024, inp_bufs=3, work_bufs=2):
    """Build + compile the per-core program. Same program on all 8 cores."""
    n_tiles = f // t
    assert n_tiles * t == f

    nc = bacc.Bacc(
        "TRN2", target_bir_lowering=False, debug=False, num_devices=N_CORES
    )

    pred = nc.dram_tensor("pred", [p, f, C], f32, kind="ExternalInput").ap()
    tgt = nc.dram_tensor("tgt", [p, f], f32, kind="ExternalInput").ap()
    pc = nc.dram_tensor("pc", [p, f], f32, kind="ExternalInput").ap()
    td = nc.dram_tensor("td", [p, f], f32, kind="ExternalInput").ap()
    ce_out = nc.dram_tensor("ce_out", [p, n_tiles], f32, kind="ExternalOutput").ap()
    w_out = nc.dram_tensor("w_out", [p, n_tiles], f32, kind="ExternalOutput").ap()
    ap_out = nc.dram_tensor("ap_out", [p, n_tiles], f32, kind="ExternalOutput").ap()
    al_out = nc.dram_tensor("al_out", [p, n_tiles], f32, kind="ExternalOutput").ap()

    with tile.TileContext(nc) as tc:
        with (
            tc.tile_pool(name="inp", bufs=inp_bufs) as inp,
            tc.tile_pool(name="work", bufs=work_bufs) as work,
            tc.tile_pool(name="acc", bufs=1) as acc,
        ):
            cecol = acc.tile([p, n_tiles], f32, tag="cecol")
            wcol = acc.tile([p, n_tiles], f32, tag="wcol")
            apcol = acc.tile([p, n_tiles], f32, tag="apcol")
            alcol = acc.tile([p, n_tiles], f32, tag="alcol")

            for k in range(n_tiles):
                sl = slice(k * t, (k + 1) * t)

                pt = inp.tile([p, t, C], f32, tag="pt")
                nc.sync.dma_start(out=pt[:], in_=pred[:, sl, :])
                tt = inp.tile([p, t], f32, tag="tt")
                nc.sync.dma_start(out=tt[:], in_=tgt[:, sl])
                pct = inp.tile([p, t], f32, tag="pct")
                nc.sync.dma_start(out=pct[:], in_=pc[:, sl])
                tdt = inp.tile([p, t], f32, tag="tdt")
                nc.sync.dma_start(out=tdt[:], in_=td[:, sl])

                # e_j = exp(pred_j), deinterleaved to unit-stride tiles (ACT)
                e0 = work.tile([p, t], f32, tag="e0")
                e1 = work.tile([p, t], f32, tag="e1")
                e2 = work.tile([p, t], f32, tag="e2")
                nc.scalar.activation(e0[:], pt[:, :, 0], AF.Exp)
                nc.scalar.activation(e1[:], pt[:, :, 1], AF.Exp)
                nc.scalar.activation(e2[:], pt[:, :, 2], AF.Exp)

                # s = e0 + e1 + e2 (DVE); lse = ln(s) (ACT)
                s01 = work.tile([p, t], f32, tag="s01")
                nc.vector.tensor_add(s01[:], e0[:], e1[:])
                s = work.tile([p, t], f32, tag="s")
                nc.vector.tensor_add(s[:], s01[:], e2[:])
                lse = work.tile([p, t], f32, tag="lse")
                nc.scalar.activation(lse[:], s[:], AF.Ln)

                # masks for target selection (GpSimd, f32-encoded targets)
                m0 = work.tile([p, t], mybir.dt.uint32, tag="m0")
                nc.gpsimd.tensor_scalar(
                    out=m0[:], in0=tt[:], scalar1=0.0, scalar2=None, op0=OP.is_equal
                )
                m2 = work.tile([p, t], mybir.dt.uint32, tag="m2")
                nc.gpsimd.tensor_scalar(
                    out=m2[:], in0=tt[:], scalar1=2.0, scalar2=None, op0=OP.is_equal
                )

                # e1 <- e[target] via predicated overwrites (DVE), then ln
                nc.vector.copy_predicated(out=e1[:], mask=m2[:], data=e2[:])
                nc.vector.copy_predicated(out=e1[:], mask=m0[:], data=e0[:])
                lsel = work.tile([p, t], f32, tag="lsel")
                nc.scalar.activation(lsel[:], e1[:], AF.Ln)

                # ap = |pc| with fused per-partition sum (ACT)
                apt = work.tile([p, t], f32, tag="apt")
                nc.scalar.activation(
                    apt[:], pct[:], AF.Abs, accum_out=apcol[:, k : k + 1]
                )

                # ce = lse - lsel, Sce accumulated; w = ce * ap, Sw accumulated
                # (scalar_tensor_tensor: out = (in0 op0 scalar) op1 in1,
                #  accum_out = sum(out); tensor_tensor_reduce crashes this HW)
                ce = work.tile([p, t], f32, tag="ce")
                nc.vector.scalar_tensor_tensor(
                    out=ce[:],
                    in0=lse[:],
                    scalar=0.0,
                    in1=lsel[:],
                    op0=OP.add,
                    op1=OP.subtract,
                    accum_out=cecol[:, k : k + 1],
                )
                w = work.tile([p, t], f32, tag="w")
                nc.vector.scalar_tensor_tensor(
                    out=w[:],
                    in0=ce[:],
                    scalar=0.0,
                    in1=apt[:],
                    op0=OP.add,
                    op1=OP.mult,
                    accum_out=wcol[:, k : k + 1],
                )

                # aligned = ((t-1)*td > 0), summed (GpSimd)
                u = work.tile([p, t], f32, tag="u")
                nc.gpsimd.tensor_scalar(
                    out=u[:], in0=tt[:], scalar1=1.0, scalar2=None, op0=OP.subtract
                )
                q = work.tile([p, t], f32, tag="q")
                nc.gpsimd.tensor_mul(q[:], u[:], tdt[:])
                al = work.tile([p, t], f32, tag="al")
                nc.vector.tensor_scalar(
                    out=al[:],
                    in0=q[:],
                    scalar1=0.0,
                    scalar2=None,
                    op0=OP.is_gt,
                    op1=OP.add,
                    accum_out=alcol[:, k : k + 1],
                )

            nc.sync.dma_start(out=ce_out[:], in_=cecol[:])
            nc.sync.dma_start(out=w_out[:], in_=wcol[:])
            nc.sync.dma_start(out=ap_out[:], in_=apcol[:])
            nc.sync.dma_start(out=al_out[:], in_=alcol[:])

    nc.compile()
    return nc


_NC = None


def _get_nc():
    global _NC
    if _NC is None:
        _NC = build()
    return _NC


def make_in_maps(predictions, targets, price_changes, trend_direction):
    predictions = np.asarray(predictions, dtype=np.float32)
    targets_f = np.asarray(targets).astype(np.float32)
    price_changes = np.asarray(price_changes, dtype=np.float32)
    trend_direction = np.asarray(trend_direction, dtype=np.float32)

    in_maps = []
    for c in range(N_CORES):
        sl = slice(c * N_PER_CORE, (c + 1) * N_PER_CORE)
        in_maps.append(
            {
                "pred": np.ascontiguousarray(predictions[sl]).reshape(P, F, C),
                "tgt": targets_f[sl].reshape(P, F),
                "pc": np.ascontiguousarray(price_changes[sl]).reshape(P, F),
                "td": np.ascontiguousarray(trend_direction[sl]).reshape(P, F),
            }
        )
    return in_maps


def combine(results):
    """Host-side reduction of per-core partial sums -> final scalar loss."""
    s_ce = s_w = s_ap = s_al = 0.0
    for r in results:
        s_ce += float(r["ce_out"].astype(np.float64).sum())
        s_w += float(r["w_out"].astype(np.float64).sum())
        s_ap += float(r["ap_out"].astype(np.float64).sum())
        s_al += float(r["al_out"].astype(np.float64).sum())

    mean_ap = s_ap / B
    weighted_ce_mean = (s_w / B) / (mean_ap + EPS)
    ce_mean = s_ce / B
    trend_mean = -0.1 * s_al / B
    loss = (
        DIRECTIONAL_WEIGHT * weighted_ce_mean
        + MAGNITUDE_WEIGHT * ce_mean
        + TREND_WEIGHT * trend_mean
    )
    return np.float32(loss)


def kernel(predictions, targets, price_changes, trend_direction):
    nc = _get_nc()
    in_maps = make_in_maps(predictions, targets, price_changes, trend_direction)
    res = run_bass_kernel_spmd(nc, in_maps, core_ids=list(range(N_CORES)))
    return combine(res.results)


# revision 11
# speedup vs baseline: 4.0242x; 4.0242x over previous
"""Trainium2 Bass kernel for CustomTradingLoss.

Computes, over B=8388608 samples with C=3 classes:
    ce      = logsumexp(pred) - pred[target]          (per sample)
    loss    = 0.85 * mean(ce * |pc|) / (mean(|pc|) + 1e-8)
            + 0.15 * mean(ce)
            + 0.1  * mean(where(aligned, -0.1, 0))
    aligned = (td > 0 & t == 2) | (td < 0 & t == 0)  == ((t-1)*td > 0)

Pure data parallel across 8 NeuronCores: core c gets samples
[c*B/8, (c+1)*B/8), laid out [128 partitions x 8192 free]. Each core
emits per-tile partial sums (Sce, Sw, Sap, Sal) as [128, n_tiles]
columns; the host reduces them in f64 and applies the final formula.
"""

import os
import sys

import numpy as np

for _p in ("/opt/trn_rl_repo", "/opt/trn_rl_repo/concourse"):
    if os.path.isdir(_p) and _p not in sys.path:
        sys.path.insert(0, _p)

import concourse.bacc as bacc
import concourse.mybir as mybir
import concourse.tile as tile
from concourse.bass_utils import run_bass_kernel_spmd


def _force_single_act_table():
    """Make both bass and walrus use natural_log_exp_and_others (covers
    exp, ln, abs, copy, relu...) as the only activation table set, as set
    id 0 on both sides. Without this, bass's first-match set chooser
    alternates exp/ln table loads every tile (~1.3us each + a bubble).

    Two halves that must stay consistent:
      - bass picks set ids from hw_specs.get_activation_tables -> patch
        bacc's binding to a single-entry dict (id 0 = the combined set)
      - walrus validates/loads ids against act_info.json -> point
        BASS_ACT_ROOT_JSON_PATH at a filtered copy with just that set
    """
    import concourse.hw_specs as hw_specs

    name = "natural_log_exp_and_others"
    tables = hw_specs.get_activation_tables("gen3")
    if name in tables:
        bacc.get_activation_tables = lambda arch: {name: tables[name]}

    if os.environ.get("BASS_ACT_ROOT_JSON_PATH"):
        return
    import glob
    import json
    import shutil
    import tempfile

    import neuronxcc

    hits = glob.glob(
        os.path.join(os.path.dirname(neuronxcc.__file__), "pwp", "*", "act_info.json")
    )
    if not hits:
        return
    src = hits[0]
    d = json.load(open(src))
    keep = [s for s in d.get("act_func_sets", []) if s.get("name") == name]
    if not keep:
        return
    tmpdir = tempfile.mkdtemp(prefix="act_single_")
    for fn in os.listdir(os.path.dirname(src)):
        srcf = os.path.join(os.path.dirname(src), fn)
        if os.path.isfile(srcf) and fn != "act_info.json":
            try:
                os.symlink(srcf, os.path.join(tmpdir, fn))
            except OSError:
                shutil.copy(srcf, os.path.join(tmpdir, fn))
    d["act_func_sets"] = keep
    with open(os.path.join(tmpdir, "act_info.json"), "w") as f:
        json.dump(d, f)
    os.environ["BASS_ACT_ROOT_JSON_PATH"] = os.path.join(tmpdir, "act_info.json")

B = 8388608
C = 3
N_CORES = 8
N_PER_CORE = B // N_CORES  # 1048576
P = 128
F = N_PER_CORE // P  # 8192 free elements per partition

DIRECTIONAL_WEIGHT = 0.85
MAGNITUDE_WEIGHT = 0.15
TREND_WEIGHT = 0.1
EPS = 1e-8

f32 = mybir.dt.float32
AF = mybir.ActivationFunctionType
OP = mybir.AluOpType


def build(p=P, f=F, t=1024, inp_bufs=3, work_bufs=2):
    """Build + compile the per-core program. Same program on all 8 cores."""
    _force_single_act_table()
    n_tiles = f // t
    assert n_tiles * t == f

    nc = bacc.Bacc(
        "TRN2", target_bir_lowering=False, debug=False, num_devices=N_CORES
    )

    pred = nc.dram_tensor("pred", [p, f, C], f32, kind="ExternalInput").ap()
    tgt = nc.dram_tensor("tgt", [p, f], f32, kind="ExternalInput").ap()
    pc = nc.dram_tensor("pc", [p, f], f32, kind="ExternalInput").ap()
    td = nc.dram_tensor("td", [p, f], f32, kind="ExternalInput").ap()
    ce_out = nc.dram_tensor("ce_out", [p, n_tiles], f32, kind="ExternalOutput").ap()
    w_out = nc.dram_tensor("w_out", [p, n_tiles], f32, kind="ExternalOutput").ap()
    ap_out = nc.dram_tensor("ap_out", [p, n_tiles], f32, kind="ExternalOutput").ap()
    al_out = nc.dram_tensor("al_out", [p, n_tiles], f32, kind="ExternalOutput").ap()

    with tile.TileContext(nc) as tc:
        with (
            tc.tile_pool(name="inp", bufs=inp_bufs) as inp,
            tc.tile_pool(name="work", bufs=work_bufs) as work,
            tc.tile_pool(name="acc", bufs=1) as acc,
        ):
            cecol = acc.tile([p, n_tiles], f32, tag="cecol")
            wcol = acc.tile([p, n_tiles], f32, tag="wcol")
            apcol = acc.tile([p, n_tiles], f32, tag="apcol")
            alcol = acc.tile([p, n_tiles], f32, tag="alcol")

            for k in range(n_tiles):
                sl = slice(k * t, (k + 1) * t)

                pt = inp.tile([p, t, C], f32, tag="pt")
                nc.sync.dma_start(out=pt[:], in_=pred[:, sl, :])
                tt = inp.tile([p, t], f32, tag="tt")
                nc.sync.dma_start(out=tt[:], in_=tgt[:, sl])
                pct = inp.tile([p, t], f32, tag="pct")
                nc.sync.dma_start(out=pct[:], in_=pc[:, sl])
                tdt = inp.tile([p, t], f32, tag="tdt")
                nc.sync.dma_start(out=tdt[:], in_=td[:, sl])

                # e_j = exp(pred_j), deinterleaved to unit-stride tiles (ACT)
                e0 = work.tile([p, t], f32, tag="e0")
                e1 = work.tile([p, t], f32, tag="e1")
                e2 = work.tile([p, t], f32, tag="e2")
                nc.scalar.activation(e0[:], pt[:, :, 0], AF.Exp)
                nc.scalar.activation(e1[:], pt[:, :, 1], AF.Exp)
                nc.scalar.activation(e2[:], pt[:, :, 2], AF.Exp)

                # s = e0 + e1 + e2 (DVE); lse = ln(s) (ACT)
                s01 = work.tile([p, t], f32, tag="s01")
                nc.vector.tensor_add(s01[:], e0[:], e1[:])
                s = work.tile([p, t], f32, tag="s")
                nc.vector.tensor_add(s[:], s01[:], e2[:])
                lse = work.tile([p, t], f32, tag="lse")
                nc.scalar.activation(lse[:], s[:], AF.Ln)

                # masks for target selection (DVE tensor_scalar, 2x mode;
                # NOTE: keep GpSimd idle — any Pool op holds the shared SBUF
                # port and stalls every 2-input DVE op for its full duration)
                m0 = work.tile([p, t], mybir.dt.uint32, tag="m0")
                nc.vector.tensor_scalar(
                    out=m0[:], in0=tt[:], scalar1=0.0, scalar2=None, op0=OP.is_equal
                )
                m2 = work.tile([p, t], mybir.dt.uint32, tag="m2")
                nc.vector.tensor_scalar(
                    out=m2[:], in0=tt[:], scalar1=2.0, scalar2=None, op0=OP.is_equal
                )

                # e1 <- e[target] via predicated overwrites (DVE), then ln
                nc.vector.copy_predicated(out=e1[:], mask=m2[:], data=e2[:])
                nc.vector.copy_predicated(out=e1[:], mask=m0[:], data=e0[:])
                lsel = work.tile([p, t], f32, tag="lsel")
                nc.scalar.activation(lsel[:], e1[:], AF.Ln)

                # ap = |pc| with fused per-partition sum (ACT)
                apt = work.tile([p, t], f32, tag="apt")
                nc.scalar.activation(
                    apt[:], pct[:], AF.Abs, accum_out=apcol[:, k : k + 1]
                )

                # ce = lse - lsel, Sce accumulated; w = ce * ap, Sw accumulated
                # (scalar_tensor_tensor: out = (in0 op0 scalar) op1 in1,
                #  accum_out = sum(out); tensor_tensor_reduce crashes this HW)
                ce = work.tile([p, t], f32, tag="ce")
                nc.vector.scalar_tensor_tensor(
                    out=ce[:],
                    in0=lse[:],
                    scalar=0.0,
                    in1=lsel[:],
                    op0=OP.add,
                    op1=OP.subtract,
                    accum_out=cecol[:, k : k + 1],
                )
                w = work.tile([p, t], f32, tag="w")
                nc.vector.scalar_tensor_tensor(
                    out=w[:],
                    in0=ce[:],
                    scalar=0.0,
                    in1=apt[:],
                    op0=OP.add,
                    op1=OP.mult,
                    accum_out=wcol[:, k : k + 1],
                )

                # aligned = ((t-1)*td > 0), summed (DVE)
                q = work.tile([p, t], f32, tag="q")
                nc.vector.scalar_tensor_tensor(
                    out=q[:],
                    in0=tt[:],
                    scalar=1.0,
                    in1=tdt[:],
                    op0=OP.subtract,
                    op1=OP.mult,
                )
                al = work.tile([p, t], f32, tag="al")
                nc.vector.tensor_scalar(
                    out=al[:],
                    in0=q[:],
                    scalar1=0.0,
                    scalar2=None,
                    op0=OP.is_gt,
                    op1=OP.add,
                    accum_out=alcol[:, k : k + 1],
                )

            nc.sync.dma_start(out=ce_out[:], in_=cecol[:])
            nc.sync.dma_start(out=w_out[:], in_=wcol[:])
            nc.sync.dma_start(out=ap_out[:], in_=apcol[:])
            nc.sync.dma_start(out=al_out[:], in_=alcol[:])

    nc.compile()
    return nc


_NC = None


def _get_nc():
    global _NC
    if _NC is None:
        _NC = build()
    return _NC


def make_in_maps(predictions, targets, price_changes, trend_direction):
    predictions = np.asarray(predictions, dtype=np.float32)
    targets_f = np.asarray(targets).astype(np.float32)
    price_changes = np.asarray(price_changes, dtype=np.float32)
    trend_direction = np.asarray(trend_direction, dtype=np.float32)

    in_maps = []
    for c in range(N_CORES):
        sl = slice(c * N_PER_CORE, (c + 1) * N_PER_CORE)
        in_maps.append(
            {
                "pred": np.ascontiguousarray(predictions[sl]).reshape(P, F, C),
                "tgt": targets_f[sl].reshape(P, F),
                "pc": np.ascontiguousarray(price_changes[sl]).reshape(P, F),
                "td": np.ascontiguousarray(trend_direction[sl]).reshape(P, F),
            }
        )
    return in_maps


def combine(results):
    """Host-side reduction of per-core partial sums -> final scalar loss."""
    s_ce = s_w = s_ap = s_al = 0.0
    for r in results:
        s_ce += float(r["ce_out"].astype(np.float64).sum())
        s_w += float(r["w_out"].astype(np.float64).sum())
        s_ap += float(r["ap_out"].astype(np.float64).sum())
        s_al += float(r["al_out"].astype(np.float64).sum())

    mean_ap = s_ap / B
    weighted_ce_mean = (s_w / B) / (mean_ap + EPS)
    ce_mean = s_ce / B
    trend_mean = -0.1 * s_al / B
    loss = (
        DIRECTIONAL_WEIGHT * weighted_ce_mean
        + MAGNITUDE_WEIGHT * ce_mean
        + TREND_WEIGHT * trend_mean
    )
    return np.float32(loss)


def kernel(predictions, targets, price_changes, trend_direction):
    nc = _get_nc()
    in_maps = make_in_maps(predictions, targets, price_changes, trend_direction)
    res = run_bass_kernel_spmd(nc, in_maps, core_ids=list(range(N_CORES)))
    return combine(res.results)


# revision 12
# speedup vs baseline: 4.6733x; 1.1613x over previous
"""Trainium2 Bass kernel for CustomTradingLoss.

Computes, over B=8388608 samples with C=3 classes:
    ce      = logsumexp(pred) - pred[target]          (per sample)
    loss    = 0.85 * mean(ce * |pc|) / (mean(|pc|) + 1e-8)
            + 0.15 * mean(ce)
            + 0.1  * mean(where(aligned, -0.1, 0))
    aligned = (td > 0 & t == 2) | (td < 0 & t == 0)  == ((t-1)*td > 0)

Pure data parallel across 8 NeuronCores: core c gets samples
[c*B/8, (c+1)*B/8), laid out [128 partitions x 8192 free]. Each core
emits per-tile partial sums (Sce, Sw, Sap, Sal) as [128, n_tiles] f32
columns; the host reduces them in f64 and applies the final formula.

The on-device datapath runs in bf16 (inputs are cast host-side):
  - halves HBM traffic (the kernel is memory-bound at f32)
  - unlocks DVE 2x/4x perf modes (fp32 tensor_tensor is capped at 1x)
Targets {0,1,2} and all signs are exact in bf16; the quantization noise
on ce is ~0.4% zero-mean per sample and averages out over 8.4M samples
(measured end-to-end rel err ~1e-4 vs the f32 reference).

Engine placement notes (hardware-measured):
  - GpSimd must stay IDLE: any Pool op holds the DVE-shared SBUF port
    for its whole (slow) duration, stalling every 2-input DVE op.
  - tensor_tensor_reduce crashes this HW; scalar_tensor_tensor with
    accum_out is the working fused multiply+reduce.
  - bass's activation-table chooser is first-match; without forcing a
    single combined exp+ln set it reloads tables every tile.
"""

import os
import sys

import numpy as np

for _p in ("/opt/trn_rl_repo", "/opt/trn_rl_repo/concourse"):
    if os.path.isdir(_p) and _p not in sys.path:
        sys.path.insert(0, _p)

import ml_dtypes

import concourse.bacc as bacc
import concourse.mybir as mybir
import concourse.tile as tile
from concourse.bass_utils import run_bass_kernel_spmd

B = 8388608
C = 3
N_CORES = 8
N_PER_CORE = B // N_CORES  # 1048576
P = 128
F = N_PER_CORE // P  # 8192 free elements per partition
T = 1024  # tile free size

DIRECTIONAL_WEIGHT = 0.85
MAGNITUDE_WEIGHT = 0.15
TREND_WEIGHT = 0.1
EPS = 1e-8

f32 = mybir.dt.float32
bf16 = mybir.dt.bfloat16
u32 = mybir.dt.uint32
AF = mybir.ActivationFunctionType
OP = mybir.AluOpType
BF16 = ml_dtypes.bfloat16


def _force_single_act_table():
    """Make both bass and walrus use natural_log_exp_and_others (covers
    exp, ln, abs, copy, relu...) as the only activation table set, as set
    id 0 on both sides. Without this, bass's first-match set chooser
    alternates exp/ln table loads every tile (~1.3us each + a bubble).

    Two halves that must stay consistent:
      - bass picks set ids from hw_specs.get_activation_tables -> patch
        bacc's binding to a single-entry dict (id 0 = the combined set)
      - walrus validates/loads ids against act_info.json -> point
        BASS_ACT_ROOT_JSON_PATH at a filtered copy with just that set
    """
    import concourse.hw_specs as hw_specs

    name = "natural_log_exp_and_others"
    tables = hw_specs.get_activation_tables("gen3")
    if name in tables:
        bacc.get_activation_tables = lambda arch: {name: tables[name]}

    if os.environ.get("BASS_ACT_ROOT_JSON_PATH"):
        return
    import glob
    import json
    import shutil
    import tempfile

    import neuronxcc

    hits = glob.glob(
        os.path.join(os.path.dirname(neuronxcc.__file__), "pwp", "*", "act_info.json")
    )
    if not hits:
        return
    src = hits[0]
    d = json.load(open(src))
    keep = [s for s in d.get("act_func_sets", []) if s.get("name") == name]
    if not keep:
        return
    tmpdir = tempfile.mkdtemp(prefix="act_single_")
    for fn in os.listdir(os.path.dirname(src)):
        srcf = os.path.join(os.path.dirname(src), fn)
        if os.path.isfile(srcf) and fn != "act_info.json":
            try:
                os.symlink(srcf, os.path.join(tmpdir, fn))
            except OSError:
                shutil.copy(srcf, os.path.join(tmpdir, fn))
    d["act_func_sets"] = keep
    with open(os.path.join(tmpdir, "act_info.json"), "w") as f:
        json.dump(d, f)
    os.environ["BASS_ACT_ROOT_JSON_PATH"] = os.path.join(tmpdir, "act_info.json")


def build(p=P, f=F, t=T, inp_bufs=3, work_bufs=2):
    """Build + compile the per-core program. Same program on all 8 cores.

    Inputs (bf16, packed host-side):
      pred [p, f, 3]           per-sample class logits (interleaved)
      aux  [p, f//t, 3, t]     per tile: [targets | price_changes | trend]
    Outputs (f32): ce_out/w_out/ap_out/al_out [p, f//t] per-tile partials.
    """
    _force_single_act_table()
    n_tiles = f // t
    assert n_tiles * t == f

    nc = bacc.Bacc(
        "TRN2", target_bir_lowering=False, debug=False, num_devices=N_CORES
    )

    pred = nc.dram_tensor("pred", [p, f, C], bf16, kind="ExternalInput").ap()
    aux = nc.dram_tensor("aux", [p, n_tiles, 3, t], bf16, kind="ExternalInput").ap()
    ce_out = nc.dram_tensor("ce_out", [p, n_tiles], f32, kind="ExternalOutput").ap()
    w_out = nc.dram_tensor("w_out", [p, n_tiles], f32, kind="ExternalOutput").ap()
    ap_out = nc.dram_tensor("ap_out", [p, n_tiles], f32, kind="ExternalOutput").ap()
    al_out = nc.dram_tensor("al_out", [p, n_tiles], f32, kind="ExternalOutput").ap()

    with tile.TileContext(nc) as tc:
        with (
            tc.tile_pool(name="inp", bufs=inp_bufs) as inp,
            tc.tile_pool(name="work", bufs=work_bufs) as work,
            tc.tile_pool(name="acc", bufs=1) as acc,
        ):
            cecol = acc.tile([p, n_tiles], f32, tag="cecol")
            wcol = acc.tile([p, n_tiles], f32, tag="wcol")
            apcol = acc.tile([p, n_tiles], f32, tag="apcol")
            alcol = acc.tile([p, n_tiles], f32, tag="alcol")

            for k in range(n_tiles):
                pt = inp.tile([p, t, C], bf16, tag="pt")
                nc.sync.dma_start(out=pt[:], in_=pred[:, k * t : (k + 1) * t, :])
                ax = inp.tile([p, 3, t], bf16, tag="ax")
                nc.sync.dma_start(out=ax[:], in_=aux[:, k, :, :])
                tt = ax[:, 0, :]
                pct = ax[:, 1, :]
                tdt = ax[:, 2, :]

                # e_j = exp(pred_j), deinterleaved to unit-stride bf16 (ACT)
                e0 = work.tile([p, t], bf16, tag="e0")
                e1 = work.tile([p, t], bf16, tag="e1")
                e2 = work.tile([p, t], bf16, tag="e2")
                nc.scalar.activation(e0[:], pt[:, :, 0], AF.Exp)
                nc.scalar.activation(e1[:], pt[:, :, 1], AF.Exp)
                nc.scalar.activation(e2[:], pt[:, :, 2], AF.Exp)

                # s = e0 + e1 + e2 (DVE bf16 2x); lse = ln(s) (ACT)
                s01 = work.tile([p, t], bf16, tag="s01")
                nc.vector.tensor_add(s01[:], e0[:], e1[:])
                s = work.tile([p, t], bf16, tag="s")
                nc.vector.tensor_add(s[:], s01[:], e2[:])
                lse = work.tile([p, t], bf16, tag="lse")
                nc.scalar.activation(lse[:], s[:], AF.Ln)

                # masks for target selection (DVE tensor_scalar;
                # GpSimd must stay idle -- see module docstring)
                m0 = work.tile([p, t], u32, tag="m0")
                nc.vector.tensor_scalar(
                    out=m0[:], in0=tt, scalar1=0.0, scalar2=None, op0=OP.is_equal
                )
                m2 = work.tile([p, t], u32, tag="m2")
                nc.vector.tensor_scalar(
                    out=m2[:], in0=tt, scalar1=2.0, scalar2=None, op0=OP.is_equal
                )

                # e1 <- e[target] via predicated overwrites (DVE), then ln
                nc.vector.copy_predicated(out=e1[:], mask=m2[:], data=e2[:])
                nc.vector.copy_predicated(out=e1[:], mask=m0[:], data=e0[:])
                lsel = work.tile([p, t], bf16, tag="lsel")
                nc.scalar.activation(lsel[:], e1[:], AF.Ln)

                # ap = |pc| with fused per-partition sum (ACT)
                apt = work.tile([p, t], bf16, tag="apt")
                nc.scalar.activation(
                    apt[:], pct, AF.Abs, accum_out=apcol[:, k : k + 1]
                )

                # ce = lse - lsel, Sce accumulated; w = ce * ap, Sw accumulated
                # (scalar_tensor_tensor: out = (in0 op0 scalar) op1 in1,
                #  accum_out = sum(out))
                ce = work.tile([p, t], bf16, tag="ce")
                nc.vector.scalar_tensor_tensor(
                    out=ce[:],
                    in0=lse[:],
                    scalar=0.0,
                    in1=lsel[:],
                    op0=OP.add,
                    op1=OP.subtract,
                    accum_out=cecol[:, k : k + 1],
                )
                w = work.tile([p, t], bf16, tag="w")
                nc.vector.scalar_tensor_tensor(
                    out=w[:],
                    in0=ce[:],
                    scalar=0.0,
                    in1=apt[:],
                    op0=OP.add,
                    op1=OP.mult,
                    accum_out=wcol[:, k : k + 1],
                )

                # aligned = ((t-1)*td > 0), summed (DVE)
                q = work.tile([p, t], bf16, tag="q")
                nc.vector.scalar_tensor_tensor(
                    out=q[:],
                    in0=tt,
                    scalar=1.0,
                    in1=tdt,
                    op0=OP.subtract,
                    op1=OP.mult,
                )
                al = work.tile([p, t], bf16, tag="al")
                nc.vector.tensor_scalar(
                    out=al[:],
                    in0=q[:],
                    scalar1=0.0,
                    scalar2=None,
                    op0=OP.is_gt,
                    op1=OP.add,
                    accum_out=alcol[:, k : k + 1],
                )

            nc.sync.dma_start(out=ce_out[:], in_=cecol[:])
            nc.sync.dma_start(out=w_out[:], in_=wcol[:])
            nc.sync.dma_start(out=ap_out[:], in_=apcol[:])
            nc.sync.dma_start(out=al_out[:], in_=alcol[:])

    nc.compile()
    return nc


_NC = None


def _get_nc():
    global _NC
    if _NC is None:
        _NC = build()
    return _NC


def make_in_maps(predictions, targets, price_changes, trend_direction, p=P, t=T):
    """Shard across cores and pack into the kernel's bf16 input layout."""
    predictions = np.asarray(predictions)
    targets = np.asarray(targets)
    price_changes = np.asarray(price_changes)
    trend_direction = np.asarray(trend_direction)

    n = predictions.shape[0]
    n_per_core = n // N_CORES
    f = n_per_core // p
    n_tiles = f // t

    pred_bf = predictions.astype(BF16)
    tgt_bf = targets.astype(BF16)
    pc_bf = price_changes.astype(BF16)
    td_bf = trend_direction.astype(BF16)

    in_maps = []
    for c in range(N_CORES):
        sl = slice(c * n_per_core, (c + 1) * n_per_core)
        aux = np.stack(
            [
                tgt_bf[sl].reshape(p, n_tiles, t),
                pc_bf[sl].reshape(p, n_tiles, t),
                td_bf[sl].reshape(p, n_tiles, t),
            ],
            axis=2,
        )  # [p, n_tiles, 3, t]
        in_maps.append(
            {
                "pred": np.ascontiguousarray(pred_bf[sl]).reshape(p, f, C),
                "aux": np.ascontiguousarray(aux),
            }
        )
    return in_maps


def combine(results):
    """Host-side reduction of per-core partial sums -> final scalar loss."""
    s_ce = s_w = s_ap = s_al = 0.0
    for r in results:
        s_ce += float(r["ce_out"].astype(np.float64).sum())
        s_w += float(r["w_out"].astype(np.float64).sum())
        s_ap += float(r["ap_out"].astype(np.float64).sum())
        s_al += float(r["al_out"].astype(np.float64).sum())

    mean_ap = s_ap / B
    weighted_ce_mean = (s_w / B) / (mean_ap + EPS)
    ce_mean = s_ce / B
    trend_mean = -0.1 * s_al / B
    loss = (
        DIRECTIONAL_WEIGHT * weighted_ce_mean
        + MAGNITUDE_WEIGHT * ce_mean
        + TREND_WEIGHT * trend_mean
    )
    return np.float32(loss)


def kernel(predictions, targets, price_changes, trend_direction):
    nc = _get_nc()
    in_maps = make_in_maps(predictions, targets, price_changes, trend_direction)
    res = run_bass_kernel_spmd(nc, in_maps, core_ids=list(range(N_CORES)))
    return combine(res.results)


# revision 14
# speedup vs baseline: 5.3464x; 1.1440x over previous
"""Trainium2 Bass kernel for CustomTradingLoss.

Computes, over B=8388608 samples with C=3 classes:
    ce      = logsumexp(pred) - pred[target]          (per sample)
    loss    = 0.85 * mean(ce * |pc|) / (mean(|pc|) + 1e-8)
            + 0.15 * mean(ce)
            + 0.1  * mean(where(aligned, -0.1, 0))
    aligned = (td > 0 & t == 2) | (td < 0 & t == 0)  == ((t-1)*td > 0)

Pure data parallel across 8 NeuronCores: core c gets samples
[c*B/8, (c+1)*B/8), laid out [128 partitions x 8192 free]. Each core
emits partial sums; the host reduces them in f64 and applies the final
formula (the three means only need global sums, so no collectives).

The on-device datapath runs in bf16 (inputs are cast host-side):
  - halves HBM traffic (the kernel is memory-bound at f32)
  - unlocks DVE 2x/4x perf modes (fp32 tensor_tensor is capped at 1x)
Targets {0,1,2} and all signs are exact in bf16; the quantization noise
on ce is ~0.4% zero-mean per sample and averages out over 8.4M samples
(measured end-to-end rel err ~1e-4 vs the f32 reference).

Engine placement notes (hardware-measured):
  - GpSimd must stay IDLE: any Pool op holds the DVE-shared SBUF port
    for its whole (slow) duration, stalling every 2-input DVE op.
  - tensor_tensor_reduce crashes this HW; sums of ce/w/al instead go
    through the otherwise-idle PE as ones-vector matmuls accumulating
    in PSUM (f32), which costs the DVE nothing.
  - bass's activation-table chooser is first-match; without forcing a
    single combined exp+ln set it reloads tables every tile.
"""

import os
import sys

import numpy as np

for _p in ("/opt/trn_rl_repo", "/opt/trn_rl_repo/concourse"):
    if os.path.isdir(_p) and _p not in sys.path:
        sys.path.insert(0, _p)

import ml_dtypes

import concourse.bacc as bacc
import concourse.mybir as mybir
import concourse.tile as tile
from concourse.bass_utils import run_bass_kernel_spmd

B = 8388608
C = 3
N_CORES = 8
N_PER_CORE = B // N_CORES  # 1048576
P = 128
F = N_PER_CORE // P  # 8192 free elements per partition
T = 2048  # tile free size

DIRECTIONAL_WEIGHT = 0.85
MAGNITUDE_WEIGHT = 0.15
TREND_WEIGHT = 0.1
EPS = 1e-8

f32 = mybir.dt.float32
bf16 = mybir.dt.bfloat16
u8 = mybir.dt.uint8
AF = mybir.ActivationFunctionType
OP = mybir.AluOpType
BF16 = ml_dtypes.bfloat16


def _force_single_act_table():
    """Make both bass and walrus use natural_log_exp_and_others (covers
    exp, ln, abs, copy, relu...) as the only activation table set, as set
    id 0 on both sides. Without this, bass's first-match set chooser
    alternates exp/ln table loads every tile (~1.3us each + a bubble).

    Two halves that must stay consistent:
      - bass picks set ids from hw_specs.get_activation_tables -> patch
        bacc's binding to a single-entry dict (id 0 = the combined set)
      - walrus validates/loads ids against act_info.json -> point
        BASS_ACT_ROOT_JSON_PATH at a filtered copy with just that set
    """
    import concourse.hw_specs as hw_specs

    name = "natural_log_exp_and_others"
    tables = hw_specs.get_activation_tables("gen3")
    if name in tables:
        bacc.get_activation_tables = lambda arch: {name: tables[name]}

    if os.environ.get("BASS_ACT_ROOT_JSON_PATH"):
        return
    import glob
    import json
    import shutil
    import tempfile

    import neuronxcc

    hits = glob.glob(
        os.path.join(os.path.dirname(neuronxcc.__file__), "pwp", "*", "act_info.json")
    )
    if not hits:
        return
    src = hits[0]
    d = json.load(open(src))
    keep = [s for s in d.get("act_func_sets", []) if s.get("name") == name]
    if not keep:
        return
    tmpdir = tempfile.mkdtemp(prefix="act_single_")
    for fn in os.listdir(os.path.dirname(src)):
        srcf = os.path.join(os.path.dirname(src), fn)
        if os.path.isfile(srcf) and fn != "act_info.json":
            try:
                os.symlink(srcf, os.path.join(tmpdir, fn))
            except OSError:
                shutil.copy(srcf, os.path.join(tmpdir, fn))
    d["act_func_sets"] = keep
    with open(os.path.join(tmpdir, "act_info.json"), "w") as f:
        json.dump(d, f)
    os.environ["BASS_ACT_ROOT_JSON_PATH"] = os.path.join(tmpdir, "act_info.json")


def build(p=P, f=F, t=T, inp_bufs=3, work_bufs=2):
    """Build + compile the per-core program. Same program on all 8 cores.

    Inputs (bf16, packed host-side):
      pred [p, f, 3]           per-sample class logits (interleaved)
      aux  [p, f//t, 3, t]     per tile: [targets | price_changes | trend]
    Outputs (f32):
      ce_out/w_out/al_out [1, nsum] column partial sums (PE/PSUM)
      ap_out [p, f//t]          per-tile |pc| partial sums (ACT accum)
    """
    _force_single_act_table()
    n_tiles = f // t
    assert n_tiles * t == f
    n_chunks = max(1, t // 512)
    chunk = t // n_chunks
    nsum = chunk  # psum column count

    nc = bacc.Bacc(
        "TRN2", target_bir_lowering=False, debug=False, num_devices=N_CORES
    )

    pred = nc.dram_tensor("pred", [p, f, C], bf16, kind="ExternalInput").ap()
    aux = nc.dram_tensor("aux", [p, n_tiles, 3, t], bf16, kind="ExternalInput").ap()
    ce_out = nc.dram_tensor("ce_out", [1, nsum], f32, kind="ExternalOutput").ap()
    w_out = nc.dram_tensor("w_out", [1, nsum], f32, kind="ExternalOutput").ap()
    al_out = nc.dram_tensor("al_out", [1, nsum], f32, kind="ExternalOutput").ap()
    ap_out = nc.dram_tensor("ap_out", [p, n_tiles], f32, kind="ExternalOutput").ap()

    with tile.TileContext(nc) as tc:
        with (
            tc.tile_pool(name="inp", bufs=inp_bufs) as inp,
            tc.tile_pool(name="work", bufs=work_bufs) as work,
            tc.tile_pool(name="acc", bufs=1) as acc,
            tc.tile_pool(name="psum", bufs=1, space="PSUM") as psum,
        ):
            apcol = acc.tile([p, n_tiles], f32, tag="apcol")
            ones = acc.tile([p, 1], bf16, tag="ones")
            nc.vector.memset(ones[:], 1.0)
            ps_ce = psum.tile([1, nsum], f32, tag="ps_ce")
            ps_w = psum.tile([1, nsum], f32, tag="ps_w")
            ps_al = psum.tile([1, nsum], f32, tag="ps_al")

            def pe_sum(ps, x, k):
                for j in range(n_chunks):
                    nc.tensor.matmul(
                        ps[:],
                        ones[:],
                        x[:, j * chunk : (j + 1) * chunk],
                        start=(k == 0 and j == 0),
                        stop=(k == n_tiles - 1 and j == n_chunks - 1),
                    )

            for k in range(n_tiles):
                pt = inp.tile([p, t, C], bf16, tag="pt")
                nc.sync.dma_start(out=pt[:], in_=pred[:, k * t : (k + 1) * t, :])
                ax = inp.tile([p, 3, t], bf16, tag="ax")
                nc.sync.dma_start(out=ax[:], in_=aux[:, k, :, :])
                tt = ax[:, 0, :]
                pct = ax[:, 1, :]
                tdt = ax[:, 2, :]

                # e_j = exp(pred_j), deinterleaved to unit-stride bf16 (ACT)
                e0 = work.tile([p, t], bf16, tag="e0")
                e1 = work.tile([p, t], bf16, tag="e1")
                e2 = work.tile([p, t], bf16, tag="e2")
                nc.scalar.activation(e0[:], pt[:, :, 0], AF.Exp)
                nc.scalar.activation(e1[:], pt[:, :, 1], AF.Exp)
                nc.scalar.activation(e2[:], pt[:, :, 2], AF.Exp)

                # s = e0 + e1 + e2 (DVE bf16 2x); lse = ln(s) (ACT)
                s01 = work.tile([p, t], bf16, tag="s01")
                nc.vector.tensor_add(s01[:], e0[:], e1[:])
                s = work.tile([p, t], bf16, tag="s")
                nc.vector.tensor_add(s[:], s01[:], e2[:])
                lse = work.tile([p, t], bf16, tag="lse")
                nc.scalar.activation(lse[:], s[:], AF.Ln)

                # masks for target selection (DVE tensor_scalar;
                # GpSimd must stay idle -- see module docstring)
                m0 = work.tile([p, t], u8, tag="m0")
                nc.vector.tensor_scalar(
                    out=m0[:], in0=tt, scalar1=0.0, scalar2=None, op0=OP.is_equal
                )
                m2 = work.tile([p, t], u8, tag="m2")
                nc.vector.tensor_scalar(
                    out=m2[:], in0=tt, scalar1=2.0, scalar2=None, op0=OP.is_equal
                )

                # e1 <- e[target] via predicated overwrites (DVE), then ln
                nc.vector.copy_predicated(out=e1[:], mask=m2[:], data=e2[:])
                nc.vector.copy_predicated(out=e1[:], mask=m0[:], data=e0[:])
                lsel = work.tile([p, t], bf16, tag="lsel")
                nc.scalar.activation(lsel[:], e1[:], AF.Ln)

                # ap = |pc| with fused per-partition sum (ACT)
                apt = work.tile([p, t], bf16, tag="apt")
                nc.scalar.activation(
                    apt[:], pct, AF.Abs, accum_out=apcol[:, k : k + 1]
                )

                # ce = lse - lsel; w = ce * ap  (DVE 2x TT; sums on PE)
                ce = work.tile([p, t], bf16, tag="ce")
                nc.vector.tensor_sub(ce[:], lse[:], lsel[:])
                w = work.tile([p, t], bf16, tag="w")
                nc.vector.tensor_mul(w[:], ce[:], apt[:])

                # aligned = ((t-1)*td > 0)  (DVE; sum on PE)
                u = work.tile([p, t], bf16, tag="u")
                nc.vector.tensor_scalar(
                    out=u[:], in0=tt, scalar1=1.0, scalar2=None, op0=OP.subtract
                )
                q = work.tile([p, t], bf16, tag="q")
                nc.vector.tensor_mul(q[:], u[:], tdt[:])
                al = work.tile([p, t], bf16, tag="al")
                nc.vector.tensor_scalar(
                    out=al[:], in0=q[:], scalar1=0.0, scalar2=None, op0=OP.is_gt
                )

                pe_sum(ps_ce, ce, k)
                pe_sum(ps_w, w, k)
                pe_sum(ps_al, al, k)

            sums = acc.tile([1, 3, nsum], f32, tag="sums")
            nc.vector.tensor_copy(out=sums[:, 0, :], in_=ps_ce[:])
            nc.vector.tensor_copy(out=sums[:, 1, :], in_=ps_w[:])
            nc.vector.tensor_copy(out=sums[:, 2, :], in_=ps_al[:])
            nc.sync.dma_start(out=ce_out[:], in_=sums[:, 0, :])
            nc.sync.dma_start(out=w_out[:], in_=sums[:, 1, :])
            nc.sync.dma_start(out=al_out[:], in_=sums[:, 2, :])
            nc.sync.dma_start(out=ap_out[:], in_=apcol[:])

    nc.compile()
    return nc


_NC = None


def _get_nc():
    global _NC
    if _NC is None:
        _NC = build()
    return _NC


def make_in_maps(predictions, targets, price_changes, trend_direction, p=P, t=T):
    """Shard across cores and pack into the kernel's bf16 input layout."""
    predictions = np.asarray(predictions)
    targets = np.asarray(targets)
    price_changes = np.asarray(price_changes)
    trend_direction = np.asarray(trend_direction)

    n = predictions.shape[0]
    n_per_core = n // N_CORES
    f = n_per_core // p
    n_tiles = f // t

    pred_bf = predictions.astype(BF16)
    tgt_bf = targets.astype(BF16)
    pc_bf = price_changes.astype(BF16)
    td_bf = trend_direction.astype(BF16)

    in_maps = []
    for c in range(N_CORES):
        sl = slice(c * n_per_core, (c + 1) * n_per_core)
        aux = np.stack(
            [
                tgt_bf[sl].reshape(p, n_tiles, t),
                pc_bf[sl].reshape(p, n_tiles, t),
                td_bf[sl].reshape(p, n_tiles, t),
            ],
            axis=2,
        )  # [p, n_tiles, 3, t]
        in_maps.append(
            {
                "pred": np.ascontiguousarray(pred_bf[sl]).reshape(p, f, C),
                "aux": np.ascontiguousarray(aux),
            }
        )
    return in_maps


def combine(results):
    """Host-side reduction of per-core partial sums -> final scalar loss."""
    s_ce = s_w = s_ap = s_al = 0.0
    for r in results:
        s_ce += float(r["ce_out"].astype(np.float64).sum())
        s_w += float(r["w_out"].astype(np.float64).sum())
        s_ap += float(r["ap_out"].astype(np.float64).sum())
        s_al += float(r["al_out"].astype(np.float64).sum())

    mean_ap = s_ap / B
    weighted_ce_mean = (s_w / B) / (mean_ap + EPS)
    ce_mean = s_ce / B
    trend_mean = -0.1 * s_al / B
    loss = (
        DIRECTIONAL_WEIGHT * weighted_ce_mean
        + MAGNITUDE_WEIGHT * ce_mean
        + TREND_WEIGHT * trend_mean
    )
    return np.float32(loss)


def kernel(predictions, targets, price_changes, trend_direction):
    nc = _get_nc()
    in_maps = make_in_maps(predictions, targets, price_changes, trend_direction)
    res = run_bass_kernel_spmd(nc, in_maps, core_ids=list(range(N_CORES)))
    return combine(res.results)


# revision 20
# speedup vs baseline: 5.5145x; 1.0315x over previous
"""Trainium2 Bass kernel for CustomTradingLoss.

Computes, over B=8388608 samples with C=3 classes:
    ce      = logsumexp(pred) - pred[target]          (per sample)
    loss    = 0.85 * mean(ce * |pc|) / (mean(|pc|) + 1e-8)
            + 0.15 * mean(ce)
            + 0.1  * mean(where(aligned, -0.1, 0))
    aligned = (td > 0 & t == 2) | (td < 0 & t == 0)  == ((t-1)*td > 0)

Pure data parallel across 8 NeuronCores: core c gets samples
[c*B/8, (c+1)*B/8), laid out [128 partitions x 8192 free]. Each core
emits partial sums; the host reduces them in f64 and applies the final
formula (the three means only need global sums, so no collectives).

The on-device datapath runs in bf16 (inputs are cast host-side):
  - halves HBM traffic (the kernel is memory-bound at f32)
  - unlocks DVE 2x/4x perf modes (fp32 tensor_tensor is capped at 1x)
Targets {0,1,2} and all signs are exact in bf16; the quantization noise
on ce is ~0.4% zero-mean per sample and averages out over 8.4M samples
(measured end-to-end rel err ~1e-4 vs the f32 reference).

Engine placement notes (hardware-measured):
  - GpSimd must stay IDLE: any Pool op holds the DVE-shared SBUF port
    for its whole (slow) duration, stalling every 2-input DVE op.
  - tensor_tensor_reduce crashes this HW; sums of ce/w/al instead go
    through the otherwise-idle PE as ones-vector matmuls accumulating
    in PSUM (f32), which costs the DVE nothing.
  - bass's activation-table chooser is first-match; without forcing a
    single combined exp+ln set it reloads tables every tile.
"""

import os
import sys

import numpy as np

for _p in ("/opt/trn_rl_repo", "/opt/trn_rl_repo/concourse"):
    if os.path.isdir(_p) and _p not in sys.path:
        sys.path.insert(0, _p)

import ml_dtypes

import concourse.bacc as bacc
import concourse.mybir as mybir
import concourse.tile as tile
from concourse.bass_utils import run_bass_kernel_spmd

B = 8388608
C = 3
N_CORES = 8
N_PER_CORE = B // N_CORES  # 1048576
P = 128
F = N_PER_CORE // P  # 8192 free elements per partition
T = 2048  # tile free size

DIRECTIONAL_WEIGHT = 0.85
MAGNITUDE_WEIGHT = 0.15
TREND_WEIGHT = 0.1
EPS = 1e-8

f32 = mybir.dt.float32
bf16 = mybir.dt.bfloat16
u16 = mybir.dt.uint16
AF = mybir.ActivationFunctionType
OP = mybir.AluOpType
BF16 = ml_dtypes.bfloat16


def _force_single_act_table():
    """Make both bass and walrus use natural_log_exp_and_others (covers
    exp, ln, abs, copy, relu...) as the only activation table set, as set
    id 0 on both sides. Without this, bass's first-match set chooser
    alternates exp/ln table loads every tile (~1.3us each + a bubble).

    Two halves that must stay consistent:
      - bass picks set ids from hw_specs.get_activation_tables -> patch
        bacc's binding to a single-entry dict (id 0 = the combined set)
      - walrus validates/loads ids against act_info.json -> point
        BASS_ACT_ROOT_JSON_PATH at a filtered copy with just that set
    """
    import concourse.hw_specs as hw_specs

    name = "natural_log_exp_and_others"
    tables = hw_specs.get_activation_tables("gen3")
    if name in tables:
        bacc.get_activation_tables = lambda arch: {name: tables[name]}

    if os.environ.get("BASS_ACT_ROOT_JSON_PATH"):
        return
    import glob
    import json
    import shutil
    import tempfile

    import neuronxcc

    hits = glob.glob(
        os.path.join(os.path.dirname(neuronxcc.__file__), "pwp", "*", "act_info.json")
    )
    if not hits:
        return
    src = hits[0]
    d = json.load(open(src))
    keep = [s for s in d.get("act_func_sets", []) if s.get("name") == name]
    if not keep:
        return
    tmpdir = tempfile.mkdtemp(prefix="act_single_")
    for fn in os.listdir(os.path.dirname(src)):
        srcf = os.path.join(os.path.dirname(src), fn)
        if os.path.isfile(srcf) and fn != "act_info.json":
            try:
                os.symlink(srcf, os.path.join(tmpdir, fn))
            except OSError:
                shutil.copy(srcf, os.path.join(tmpdir, fn))
    d["act_func_sets"] = keep
    with open(os.path.join(tmpdir, "act_info.json"), "w") as f:
        json.dump(d, f)
    os.environ["BASS_ACT_ROOT_JSON_PATH"] = os.path.join(tmpdir, "act_info.json")


def build(p=P, f=F, t=T, inp_bufs=3, work_bufs=2):
    """Build + compile the per-core program. Same program on all 8 cores.

    Inputs (bf16, packed host-side):
      pred [p, f, 3]           per-sample class logits (interleaved)
      aux  [p, f//t, 3, t]     per tile: [targets | price_changes | trend]
    Outputs (f32):
      ce_out/w_out/al_out [1, nsum] column partial sums (PE/PSUM)
      ap_out [p, f//t]          per-tile |pc| partial sums (ACT accum)
    """
    _force_single_act_table()
    n_tiles = f // t
    assert n_tiles * t == f
    n_chunks = max(1, t // 512)
    chunk = t // n_chunks
    nsum = chunk  # psum column count

    nc = bacc.Bacc(
        "TRN2", target_bir_lowering=False, debug=False, num_devices=N_CORES
    )

    pred = nc.dram_tensor("pred", [p, f, C], bf16, kind="ExternalInput").ap()
    aux = nc.dram_tensor("aux", [p, n_tiles, 3, t], bf16, kind="ExternalInput").ap()
    ce_out = nc.dram_tensor("ce_out", [1, nsum], f32, kind="ExternalOutput").ap()
    w_out = nc.dram_tensor("w_out", [1, nsum], f32, kind="ExternalOutput").ap()
    al_out = nc.dram_tensor("al_out", [1, nsum], f32, kind="ExternalOutput").ap()
    ap_out = nc.dram_tensor("ap_out", [1, nsum], f32, kind="ExternalOutput").ap()

    with tile.TileContext(nc) as tc:
        with (
            tc.tile_pool(name="inp", bufs=inp_bufs) as inp,
            tc.tile_pool(name="work", bufs=work_bufs) as work,
            tc.tile_pool(name="acc", bufs=1) as acc,
            tc.tile_pool(name="psum", bufs=1, space="PSUM") as psum,
        ):
            ones = acc.tile([p, 1], bf16, tag="ones")
            nc.vector.memset(ones[:], 1.0)
            ps_ce = psum.tile([1, nsum], f32, tag="ps_ce")
            ps_w = psum.tile([1, nsum], f32, tag="ps_w")
            ps_al = psum.tile([1, nsum], f32, tag="ps_al")
            ps_ap = psum.tile([1, nsum], f32, tag="ps_ap")

            def pe_sum(ps, x, k):
                for j in range(n_chunks):
                    nc.tensor.matmul(
                        ps[:],
                        ones[:],
                        x[:, j * chunk : (j + 1) * chunk],
                        start=(k == 0 and j == 0),
                        stop=(k == n_tiles - 1 and j == n_chunks - 1),
                    )

            for k in range(n_tiles):
                pt = inp.tile([p, t, C], bf16, tag="pt")
                nc.sync.dma_start(out=pt[:], in_=pred[:, k * t : (k + 1) * t, :])
                ax = inp.tile([p, 3, t], bf16, tag="ax")
                nc.sync.dma_start(out=ax[:], in_=aux[:, k, :, :])
                tt = ax[:, 0, :]
                pct = ax[:, 1, :]
                tdt = ax[:, 2, :]

                # e_j = exp(pred_j), deinterleaved to unit-stride bf16 (ACT)
                e0 = work.tile([p, t], bf16, tag="e0")
                e1 = work.tile([p, t], bf16, tag="e1")
                e2 = work.tile([p, t], bf16, tag="e2")
                nc.scalar.activation(e0[:], pt[:, :, 0], AF.Exp)
                nc.scalar.activation(e1[:], pt[:, :, 1], AF.Exp)
                nc.scalar.activation(e2[:], pt[:, :, 2], AF.Exp)

                # s = e0 + e1 + e2 (DVE bf16 2x); lse = ln(s) (ACT)
                s01 = work.tile([p, t], bf16, tag="s01")
                nc.vector.tensor_add(s01[:], e0[:], e1[:])
                s = work.tile([p, t], bf16, tag="s")
                nc.vector.tensor_add(s[:], s01[:], e2[:])
                lse = work.tile([p, t], bf16, tag="lse")
                nc.scalar.activation(lse[:], s[:], AF.Ln)

                # masks for target selection: bf16 is_equal runs at DVE 4x;
                # the 1.0/0.0 bf16 pattern bitcasts to a valid uint16
                # predicate for copy_predicated. (GpSimd must stay idle --
                # see module docstring)
                m0 = work.tile([p, t], bf16, tag="m0")
                nc.vector.tensor_scalar(
                    out=m0[:], in0=tt, scalar1=0.0, scalar2=None, op0=OP.is_equal
                )
                m2 = work.tile([p, t], bf16, tag="m2")
                nc.vector.tensor_scalar(
                    out=m2[:], in0=tt, scalar1=2.0, scalar2=None, op0=OP.is_equal
                )

                # e1 <- e[target] via predicated overwrites (DVE), then ln
                nc.vector.copy_predicated(
                    out=e1[:], mask=m2[:].bitcast(u16), data=e2[:]
                )
                nc.vector.copy_predicated(
                    out=e1[:], mask=m0[:].bitcast(u16), data=e0[:]
                )
                lsel = work.tile([p, t], bf16, tag="lsel")
                nc.scalar.activation(lsel[:], e1[:], AF.Ln)

                # ap = |pc| by clearing the bf16 sign bit (DVE 4x int op;
                # cheaper than an ACT Abs pass). Sum goes through PE.
                apb = work.tile([p, t], mybir.dt.uint16, tag="apb")
                nc.vector.tensor_scalar(
                    out=apb[:],
                    in0=pct.bitcast(mybir.dt.uint16),
                    scalar1=0x7FFF,
                    scalar2=None,
                    op0=OP.bitwise_and,
                )
                apt = apb[:].bitcast(bf16)

                # ce = lse - lsel; w = ce * ap  (DVE 2x TT; sums on PE)
                ce = work.tile([p, t], bf16, tag="ce")
                nc.vector.tensor_sub(ce[:], lse[:], lsel[:])
                w = work.tile([p, t], bf16, tag="w")
                nc.vector.tensor_mul(w[:], ce[:], apt)

                # aligned = ((t-1)*td > 0)  (DVE; sum on PE)
                u = work.tile([p, t], bf16, tag="u")
                nc.vector.tensor_scalar(
                    out=u[:], in0=tt, scalar1=1.0, scalar2=None, op0=OP.subtract
                )
                q = work.tile([p, t], bf16, tag="q")
                nc.vector.tensor_mul(q[:], u[:], tdt[:])
                al = work.tile([p, t], bf16, tag="al")
                nc.vector.tensor_scalar(
                    out=al[:], in0=q[:], scalar1=0.0, scalar2=None, op0=OP.is_gt
                )

                pe_sum(ps_ce, ce[:], k)
                pe_sum(ps_w, w[:], k)
                pe_sum(ps_al, al[:], k)
                pe_sum(ps_ap, apt, k)

            sums = acc.tile([1, 4, nsum], f32, tag="sums")
            nc.vector.tensor_copy(out=sums[:, 0, :], in_=ps_ce[:])
            nc.vector.tensor_copy(out=sums[:, 1, :], in_=ps_w[:])
            nc.vector.tensor_copy(out=sums[:, 2, :], in_=ps_al[:])
            nc.vector.tensor_copy(out=sums[:, 3, :], in_=ps_ap[:])
            nc.sync.dma_start(out=ce_out[:], in_=sums[:, 0, :])
            nc.sync.dma_start(out=w_out[:], in_=sums[:, 1, :])
            nc.sync.dma_start(out=al_out[:], in_=sums[:, 2, :])
            nc.sync.dma_start(out=ap_out[:], in_=sums[:, 3, :])

    nc.compile()
    return nc


_NC = None


def _get_nc():
    global _NC
    if _NC is None:
        _NC = build()
    return _NC


def make_in_maps(predictions, targets, price_changes, trend_direction, p=P, t=T):
    """Shard across cores and pack into the kernel's bf16 input layout."""
    predictions = np.asarray(predictions)
    targets = np.asarray(targets)
    price_changes = np.asarray(price_changes)
    trend_direction = np.asarray(trend_direction)

    n = predictions.shape[0]
    n_per_core = n // N_CORES
    f = n_per_core // p
    n_tiles = f // t

    pred_bf = predictions.astype(BF16)
    tgt_bf = targets.astype(BF16)
    pc_bf = price_changes.astype(BF16)
    td_bf = trend_direction.astype(BF16)

    in_maps = []
    for c in range(N_CORES):
        sl = slice(c * n_per_core, (c + 1) * n_per_core)
        aux = np.stack(
            [
                tgt_bf[sl].reshape(p, n_tiles, t),
                pc_bf[sl].reshape(p, n_tiles, t),
                td_bf[sl].reshape(p, n_tiles, t),
            ],
            axis=2,
        )  # [p, n_tiles, 3, t]
        in_maps.append(
            {
                "pred": np.ascontiguousarray(pred_bf[sl]).reshape(p, f, C),
                "aux": np.ascontiguousarray(aux),
            }
        )
    return in_maps


def combine(results):
    """Host-side reduction of per-core partial sums -> final scalar loss."""
    s_ce = s_w = s_ap = s_al = 0.0
    for r in results:
        s_ce += float(r["ce_out"].astype(np.float64).sum())
        s_w += float(r["w_out"].astype(np.float64).sum())
        s_ap += float(r["ap_out"].astype(np.float64).sum())
        s_al += float(r["al_out"].astype(np.float64).sum())

    mean_ap = s_ap / B
    weighted_ce_mean = (s_w / B) / (mean_ap + EPS)
    ce_mean = s_ce / B
    trend_mean = -0.1 * s_al / B
    loss = (
        DIRECTIONAL_WEIGHT * weighted_ce_mean
        + MAGNITUDE_WEIGHT * ce_mean
        + TREND_WEIGHT * trend_mean
    )
    return np.float32(loss)


def kernel(predictions, targets, price_changes, trend_direction):
    nc = _get_nc()
    in_maps = make_in_maps(predictions, targets, price_changes, trend_direction)
    res = run_bass_kernel_spmd(nc, in_maps, core_ids=list(range(N_CORES)))
    return combine(res.results)
